# revision 3
# baseline (speedup 1.0000x reference)
"""Trainium2 Bass kernel for nn_Decoder_AdptiveVisualCenter_codebook.

Reference computation (B=16, C=256, H=W=64, CR=32, K=16):
    xe  = conv1x1(x, 256->32) ; conv3x3(32->32, pad 1) ; conv1x1(32->256)
    xc  = conv3x3(xe, 256->256, pad 1) ; BN(inference) ; ReLU
    xf  = xc as [b, n=4096, c]
    softmax-weighted codebook aggregation e_k = einsum(weights, xf) ; e_k.mean(1)
    e   = sigmoid(fc(e_k.mean(1)))
    out = x + x * e[:, :, None, None]

Two mathematical simplifications used here:
  1. softmax weights sum to 1 over K, so
         e_k.mean(axis=1)[b, c] = (1/K) * sum_n xf[b, n, c]
     -- the codebook / distances / softmax cancel out of the final output
     exactly (up to f32 rounding).
  2. conv3 (1x1) composes with the 3x3 cbr conv into a single 3x3 conv with
     Cin=32: W_eff[o,i,t] = sum_m cbr_w[o,m,t] * conv3_w[m,i].  This is exact
     when conv3_b == 0 (true for the reference inputs); a numpy fallback
     handles the general case.

Sharding: data-parallel over batch, 2 batch elements per core x 8 cores.
Weights are replicated (tiny).

Conv matmuls run in float32r (PE full rate, ~12-bit mantissa; measured
~17x more accurate than bf16).  f32r tiles must be produced by compute
ops (ACT/DVE round on write); raw DMA bits into an f32r matmul are
rejected by the BIR verifier and fault the hardware.
"""

import os
import sys

import numpy as np

for _p in ("/opt/trn_rl_repo",):
    if _p not in sys.path:
        sys.path.insert(0, _p)

from concourse import bacc, mybir, tile
import concourse.bass as bass
from concourse.bass_utils import run_bass_kernel_spmd

N_CORES = 8
B, C, H, W = 16, 256, 64, 64
HW = H * W
CR = 32
K = 16
BN_EPS = 1e-5
BL = B // N_CORES  # batches per core
PADW = W + 2  # 66
PADHW = PADW * (H + 2)  # 4356
NCH = 8  # n-chunks of 512 spatial positions (8 image rows each)
RPC = H // NCH  # rows per chunk = 8

# conv matmul dtype mode: 'f32r' (full-rate, ~12-bit mantissa),
# 'bf16' (full-rate, 8-bit), 'f32' (4x slower, exact)
MODE = os.environ.get("KERNEL_MODE", "f32r")
# pack 3 vertical conv taps into K=96 via row-shifted SBUF replicas
PACK = os.environ.get("KERNEL_PACK", "1") == "1"
# interleave tiny bf16 matmuls to keep the PE HAM clock-gate warm
# (f32r matmuls don't register as PE activity, leaving the PE at 1.2 GHz)
HEAT = int(os.environ.get("KERNEL_HEAT", "0"))

f32 = mybir.dt.float32
f32r = mybir.dt.float32r
bf16 = mybir.dt.bfloat16
u32 = mybir.dt.uint32

TAPS = [(dy, dx) for dy in range(3) for dx in range(3)]

_NC_CACHE = {}


def _emit_kernel(tc, mode):
    nc = tc.nc
    cdt = {"f32r": f32r, "bf16": bf16, "f32": f32}[mode]
    x_d = nc.dram_tensor("x", [BL, C, HW], f32, kind="ExternalInput").ap()
    w1_d = nc.dram_tensor("w1T", [128, 2 * CR], f32, kind="ExternalInput").ap()
    w2_d = nc.dram_tensor("w2T", [CR, 9 * CR], f32, kind="ExternalInput").ap()
    wf_d = nc.dram_tensor("wfT", [CR, 9 * C], f32, kind="ExternalInput").ap()
    fc_d = nc.dram_tensor("fcT", [128, 2 * C], f32, kind="ExternalInput").ap()
    b1_d = nc.dram_tensor("b1", [CR, 1], f32, kind="ExternalInput").ap()
    b2_d = nc.dram_tensor("b2", [CR, 1], f32, kind="ExternalInput").ap()
    bns_d = nc.dram_tensor("bns", [128, 2], f32, kind="ExternalInput").ap()
    bnh_d = nc.dram_tensor("bnh", [128, 2], f32, kind="ExternalInput").ap()
    fcb_d = nc.dram_tensor("fcb", [128, 2], f32, kind="ExternalInput").ap()
    out_d = nc.dram_tensor("out", [BL, C, HW], f32, kind="ExternalOutput").ap()

    Ident = mybir.ActivationFunctionType.Identity
    Relu = mybir.ActivationFunctionType.Relu
    Sigmoid = mybir.ActivationFunctionType.Sigmoid

    import contextlib

    with contextlib.ExitStack() as ctx:
        wpool = ctx.enter_context(tc.tile_pool(name="weights", bufs=1))
        xpool = ctx.enter_context(tc.tile_pool(name="x", bufs=2))
        padpool = ctx.enter_context(tc.tile_pool(name="pads", bufs=1))
        scrpool = ctx.enter_context(tc.tile_pool(name="scratch", bufs=2))
        smpool = ctx.enter_context(tc.tile_pool(name="sums", bufs=2))
        pp_small = ctx.enter_context(tc.tile_pool(name="ps", bufs=2, space="PSUM"))
        pp_big = ctx.enter_context(tc.tile_pool(name="pb", bufs=4, space="PSUM"))
        pp_fc = ctx.enter_context(tc.tile_pool(name="pfc", bufs=1, space="PSUM"))

        def load_weight(name, dram_ap, shape):
            t_f = wpool.tile(list(shape), f32, name=f"{name}_f32")
            nc.sync.dma_start(out=t_f[:], in_=dram_ap)
            if cdt == f32:
                return t_f
            t_c = wpool.tile(list(shape), cdt, name=f"{name}_c")
            nc.vector.tensor_copy(t_c[:], t_f[:])
            return t_c

        w1_sb = load_weight("w1", w1_d, (128, 2 * CR))
        w2_sb = load_weight("w2", w2_d, (CR, 9 * CR))
        wf_sb = load_weight("wf", wf_d, (CR, 9 * C))
        fc_sb = wpool.tile([128, 2 * C], f32)
        nc.sync.dma_start(out=fc_sb[:], in_=fc_d)
        b1_sb = wpool.tile([CR, 1], f32)
        nc.sync.dma_start(out=b1_sb[:], in_=b1_d)
        b2_sb = wpool.tile([CR, 1], f32)
        nc.sync.dma_start(out=b2_sb[:], in_=b2_d)
        bns_sb = wpool.tile([128, 2], f32)
        nc.sync.dma_start(out=bns_sb[:], in_=bns_d)
        bnh_sb = wpool.tile([128, 2], f32)
        nc.sync.dma_start(out=bnh_sb[:], in_=bnh_d)
        fcb_sb = wpool.tile([128, 2], f32)
        nc.sync.dma_start(out=fcb_sb[:], in_=fcb_d)

        # --- padded intermediates (borders stay zero across batches) ---
        def zeroed_pad(name):
            t = padpool.tile([CR, PADHW], cdt, name=name)
            if cdt == f32r:
                nc.vector.memset(t[:].bitcast(u32), 0)
            else:
                nc.vector.memset(t[:], 0.0)
            return t

        pad1 = zeroed_pad("pad1")
        pad2 = zeroed_pad("pad2")
        pad1_3d = pad1.rearrange("p (r c) -> p r c", r=H + 2)
        pad2_3d = pad2.rearrange("p (r c) -> p r c", r=H + 2)

        for b in range(BL):
            # --- load x (both channel chunks) ---
            x_sb = []
            for cc in range(2):
                xt = xpool.tile([128, HW], f32, tag=f"x{cc}", name=f"x{cc}_{b}")
                nc.sync.dma_start(out=xt[:], in_=x_d[b, cc * 128:(cc + 1) * 128, :])
                x_sb.append(xt)

            if cdt == f32:
                xr = x_sb
            else:
                xr = []
                for cc in range(2):
                    xb = xpool.tile([128, HW], cdt, tag=f"xc{cc}", name=f"xc{cc}_{b}")
                    nc.vector.tensor_copy(xb[:], x_sb[cc][:])
                    xr.append(xb)

            # --- conv1: 1x1, 256 -> 32, write into pad1 interior ---
            for r in range(NCH):
                p1 = pp_small.tile([CR, 512], f32, tag="psmall", name=f"p1_{b}_{r}")
                for cc in range(2):
                    nc.tensor.matmul(
                        p1[:],
                        lhsT=w1_sb[:, cc * CR:(cc + 1) * CR],
                        rhs=xr[cc][:, r * 512:(r + 1) * 512],
                        start=(cc == 0),
                        stop=(cc == 1),
                    )
                dest = pad1_3d[:, r * RPC + 1:r * RPC + 9, 1:W + 1]
                nc.scalar.activation(
                    dest, p1.rearrange("p (a b) -> p a b", a=RPC),
                    Ident, bias=b1_sb[:], scale=1.0,
                )

            # --- conv2: 3x3, 32 -> 32, write into pad2 interior ---
            for r in range(NCH):
                p2 = pp_small.tile([CR, 512], f32, tag="psmall", name=f"p2_{b}_{r}")
                for t, (dy, dx) in enumerate(TAPS):
                    nc.tensor.matmul(
                        p2[:],
                        lhsT=w2_sb[:, t * CR:(t + 1) * CR],
                        rhs=pad1_3d[:, r * RPC + dy:r * RPC + dy + 8, dx:dx + W],
                        start=(t == 0),
                        stop=(t == 8),
                    )
                dest = pad2_3d[:, r * RPC + 1:r * RPC + 9, 1:W + 1]
                nc.scalar.activation(
                    dest, p2.rearrange("p (a b) -> p a b", a=RPC),
                    Ident, bias=b2_sb[:], scale=1.0,
                )

            # --- fused conv3+cbr: 3x3, 32 -> 256, BN+ReLU, row-sum accumulate ---
            epre = []
            for mc in range(2):
                sums = smpool.tile([128, NCH], f32, tag=f"sums{mc}",
                                   name=f"sums{mc}_{b}")
                for r in range(NCH):
                    pf = pp_big.tile([128, 512], f32, tag="pf", name=f"pf{mc}_{b}_{r}")
                    for t, (dy, dx) in enumerate(TAPS):
                        nc.tensor.matmul(
                            pf[:],
                            lhsT=wf_sb[:, t * C + mc * 128:t * C + mc * 128 + 128],
                            rhs=pad2_3d[:, r * RPC + dy:r * RPC + dy + 8, dx:dx + W],
                            start=(t == 0),
                            stop=(t == 8),
                        )
                    scr = scrpool.tile([128, 512], f32, tag="scr",
                                       name=f"scr{mc}_{b}_{r}")
                    nc.scalar.activation(
                        scr[:], pf[:], Relu,
                        bias=bnh_sb[:, mc:mc + 1], scale=bns_sb[:, mc:mc + 1],
                        accum_out=sums[:, r:r + 1],
                    )
                ep = smpool.tile([128, 1], f32, tag=f"epre{mc}", name=f"ep{mc}_{b}")
                nc.vector.tensor_reduce(
                    ep[:], sums[:], axis=mybir.AxisListType.X, op=mybir.AluOpType.add)
                epre.append(ep)

            # --- fc + sigmoid + gate ---
            for mc in range(2):
                pe = pp_fc.tile([128, 1], f32, tag="pe", name=f"pe{mc}_{b}")
                for k in range(2):
                    nc.tensor.matmul(
                        pe[:],
                        lhsT=fc_sb[:, k * C + mc * 128:k * C + mc * 128 + 128],
                        rhs=epre[k][:],
                        start=(k == 0),
                        stop=(k == 1),
                    )
                g = smpool.tile([128, 1], f32, tag=f"g{mc}", name=f"g{mc}_{b}")
                nc.scalar.activation(g[:], pe[:], Sigmoid,
                                     bias=fcb_sb[:, mc:mc + 1], scale=1.0)
                nc.vector.tensor_scalar_add(g[:], g[:], 1.0)
                nc.vector.tensor_scalar_mul(x_sb[mc][:], x_sb[mc][:], g[:])
                nc.sync.dma_start(out=out_d[b, mc * 128:(mc + 1) * 128, :],
                                  in_=x_sb[mc][:])


def _build_nc(mode):
    if mode in _NC_CACHE:
        return _NC_CACHE[mode]
    nc = bacc.Bacc("TRN2", target_bir_lowering=False, debug=False,
                   num_devices=N_CORES)
    with tile.TileContext(nc) as tc:
        _emit_kernel(tc, mode)
    nc.compile()
    _NC_CACHE[mode] = nc
    return nc


def _to_np(a, dtype=np.float32):
    return np.asarray(a, dtype=dtype)


def _prep_weights(inputs, mode):
    """Host-side weight folding. All arrays f32; on-chip casts as needed."""
    conv1_w = _to_np(inputs["conv1_w"], np.float64)  # [CR, C, 1, 1]
    conv1_b = _to_np(inputs["conv1_b"], np.float64)
    conv2_w = _to_np(inputs["conv2_w"], np.float64)  # [CR, CR, 3, 3]
    conv2_b = _to_np(inputs["conv2_b"], np.float64)
    conv3_w = _to_np(inputs["conv3_w"], np.float64)  # [C, CR, 1, 1]
    cbr_w = _to_np(inputs["cbr_w"], np.float64)      # [C, C, 3, 3]
    cbr_b = _to_np(inputs["cbr_b"], np.float64)
    bn_gamma = _to_np(inputs["bn_gamma"], np.float64)
    bn_beta = _to_np(inputs["bn_beta"], np.float64)
    bn_mean = _to_np(inputs["bn_mean"], np.float64)
    bn_var = _to_np(inputs["bn_var"], np.float64)
    fc_w = _to_np(inputs["fc_w"], np.float64)        # [C, C]
    fc_b = _to_np(inputs["fc_b"], np.float64)

    # conv1 lhsT: [K=256 (2x128 chunks), M=32] -> [128, 2*32]
    w1 = conv1_w.reshape(CR, C).T  # [C, CR]
    w1T = np.ascontiguousarray(
        w1.reshape(2, 128, CR).transpose(1, 0, 2).reshape(128, 2 * CR)
    ).astype(np.float32)

    # conv2 lhsT per tap: [i, (dy,dx), o] -> [32, 9*32]
    w2T = np.ascontiguousarray(
        conv2_w.transpose(1, 2, 3, 0).reshape(CR, 9 * CR)).astype(np.float32)

    # fused conv3+cbr lhsT per tap: W_eff[o,i,t] = sum_m cbr_w[o,m,t] conv3_w[m,i]
    c3 = conv3_w.reshape(C, CR)  # [m, i]
    w_eff = np.einsum("omyx,mi->iyxo", cbr_w, c3)  # [i, 3, 3, o]
    wfT = np.ascontiguousarray(w_eff.reshape(CR, 9 * C)).astype(np.float32)

    # BN fold: relu(psum * s + h)
    inv = 1.0 / np.sqrt(bn_var + BN_EPS)
    scale = bn_gamma * inv
    shift = (cbr_b - bn_mean) * scale + bn_beta
    bns = np.ascontiguousarray(scale.reshape(2, 128).T).astype(np.float32)
    bnh = np.ascontiguousarray(shift.reshape(2, 128).T).astype(np.float32)

    # fc lhsT with 1/K mean folded in: [K=256 (2x128), M=256] -> [128, 2*256]
    fcT = np.ascontiguousarray(
        (fc_w.T / K).reshape(2, 128, C).transpose(1, 0, 2).reshape(128, 2 * C)
    ).astype(np.float32)
    fcb = np.ascontiguousarray(fc_b.reshape(2, 128).T).astype(np.float32)

    return {
        "w1T": w1T,
        "w2T": w2T,
        "wfT": wfT,
        "fcT": fcT,
        "b1": conv1_b.reshape(CR, 1).astype(np.float32),
        "b2": conv2_b.reshape(CR, 1).astype(np.float32),
        "bns": bns,
        "bnh": bnh,
        "fcb": fcb,
    }


def _numpy_fallback(inputs):
    """Exact-path fallback (used only when conv3_b != 0, which the reference
    inputs never hit)."""
    x = _to_np(inputs["x"])  # [B, C, H, W]

    def conv1x1(xx, w, bias):
        co, ci = w.shape[0], w.shape[1]
        return (np.einsum("oi,bihw->bohw", w.reshape(co, ci), xx)
                + bias[None, :, None, None]).astype(np.float32)

    def conv3x3(xx, w, bias):
        bb, ci, hh, ww = xx.shape
        co = w.shape[0]
        xp = np.zeros((bb, ci, hh + 2, ww + 2), np.float32)
        xp[:, :, 1:-1, 1:-1] = xx
        y = np.zeros((bb, co, hh, ww), np.float32)
        for dy in range(3):
            for dx in range(3):
                y += np.einsum("oi,bihw->bohw", w[:, :, dy, dx],
                               xp[:, :, dy:dy + hh, dx:dx + ww])
        return y + bias[None, :, None, None]

    xe = conv1x1(x, _to_np(inputs["conv1_w"]), _to_np(inputs["conv1_b"]))
    xe = conv3x3(xe, _to_np(inputs["conv2_w"]), _to_np(inputs["conv2_b"]))
    xe = conv1x1(xe, _to_np(inputs["conv3_w"]), _to_np(inputs["conv3_b"]))
    xc = conv3x3(xe, _to_np(inputs["cbr_w"]), _to_np(inputs["cbr_b"]))
    inv = 1.0 / np.sqrt(_to_np(inputs["bn_var"]) + BN_EPS)
    xc = (xc - _to_np(inputs["bn_mean"])[None, :, None, None]) \
        * (_to_np(inputs["bn_gamma"]) * inv)[None, :, None, None] \
        + _to_np(inputs["bn_beta"])[None, :, None, None]
    xc = np.maximum(xc, 0.0)
    e_pre = xc.reshape(B, C, HW).sum(axis=2) / K  # softmax-sums-to-1 identity
    z = e_pre @ _to_np(inputs["fc_w"]).T + _to_np(inputs["fc_b"])
    e = 1.0 / (1.0 + np.exp(-z))
    return (x + x * e[:, :, None, None]).astype(np.float32)


def kernel(**inputs):
    if np.any(np.asarray(inputs["conv3_b"], np.float32)):
        return _numpy_fallback(inputs)

    mode = MODE
    x = _to_np(inputs["x"]).reshape(B, C, HW)
    weights = _prep_weights(inputs, mode)

    nc = _build_nc(mode)
    in_maps = []
    for c in range(N_CORES):
        m = dict(weights)
        m["x"] = np.ascontiguousarray(x[c * BL:(c + 1) * BL])
        in_maps.append(m)

    res = run_bass_kernel_spmd(nc, in_maps, list(range(N_CORES)))
    out = np.concatenate([res.results[c]["out"] for c in range(N_CORES)], axis=0)
    return out.reshape(B, C, H, W).astype(np.float32)


# revision 9
# speedup vs baseline: 1.8462x; 1.8462x over previous
"""Trainium2 Bass kernel for nn_Decoder_AdptiveVisualCenter_codebook.

Reference computation (B=16, C=256, H=W=64, CR=32, K=16):
    xe  = conv1x1(x, 256->32) ; conv3x3(32->32, pad 1) ; conv1x1(32->256)
    xc  = conv3x3(xe, 256->256, pad 1) ; BN(inference) ; ReLU
    xf  = xc as [b, n=4096, c]
    softmax-weighted codebook aggregation e_k = einsum(weights, xf) ; e_k.mean(1)
    e   = sigmoid(fc(e_k.mean(1)))
    out = x + x * e[:, :, None, None]

Two mathematical simplifications used here:
  1. softmax weights sum to 1 over K, so
         e_k.mean(axis=1)[b, c] = (1/K) * sum_n xf[b, n, c]
     -- the codebook / distances / softmax cancel out of the final output
     exactly (up to f32 rounding).
  2. conv3 (1x1) composes with the 3x3 cbr conv into a single 3x3 conv with
     Cin=32: W_eff[o,i,t] = sum_m cbr_w[o,m,t] * conv3_w[m,i].  This is exact
     when conv3_b == 0 (true for the reference inputs); a numpy fallback
     handles the general case.

Sharding: data-parallel over batch, 2 batch elements per core x 8 cores.
Weights are replicated (tiny).

Conv matmuls run in float32r (PE full rate, ~12-bit mantissa; measured
~17x more accurate than bf16).  f32r tiles must be produced by compute
ops (ACT/DVE round on write); raw DMA bits into an f32r matmul are
rejected by the BIR verifier and fault the hardware.
"""

import os
import sys

import numpy as np

for _p in ("/opt/trn_rl_repo",):
    if _p not in sys.path:
        sys.path.insert(0, _p)

from concourse import bacc, mybir, tile
import concourse.bass as bass
from concourse.bass_utils import run_bass_kernel_spmd

N_CORES = 8
B, C, H, W = 16, 256, 64, 64
HW = H * W
CR = 32
K = 16
BN_EPS = 1e-5
BL = B // N_CORES  # batches per core
PADW = W + 2  # 66
PADHW = PADW * (H + 2)  # 4356
NCH = 8  # n-chunks of 512 spatial positions (8 image rows each)
RPC = H // NCH  # rows per chunk = 8

# conv matmul dtype mode: 'f32r' (full-rate, ~12-bit mantissa),
# 'bf16' (full-rate, 8-bit), 'f32' (4x slower, exact)
MODE = os.environ.get("KERNEL_MODE", "f32r")
# pack 3 vertical conv taps into K=96 via row-shifted SBUF replicas
# (this box throttles the PE to 1.2 GHz under sustained matmul load, so
# wall time is proportional to total matmul columns -- fewer, K-fatter
# matmuls win)
PACK = os.environ.get("KERNEL_PACK", "1") == "1"

f32 = mybir.dt.float32
f32r = mybir.dt.float32r
bf16 = mybir.dt.bfloat16
u32 = mybir.dt.uint32

TAPS = [(dy, dx) for dy in range(3) for dx in range(3)]

_NC_CACHE = {}


def _emit_kernel(tc, mode):
    nc = tc.nc
    cdt = {"f32r": f32r, "bf16": bf16, "f32": f32}[mode]
    x_d = nc.dram_tensor("x", [BL, C, HW], f32, kind="ExternalInput").ap()
    w1_d = nc.dram_tensor("w1T", [128, 2 * CR], f32, kind="ExternalInput").ap()
    if PACK:
        w2_d = nc.dram_tensor("w2T", [3 * CR, 3 * CR], f32, kind="ExternalInput").ap()
        wf_d = nc.dram_tensor("wfT", [3 * CR, 3 * C], f32, kind="ExternalInput").ap()
    else:
        w2_d = nc.dram_tensor("w2T", [CR, 9 * CR], f32, kind="ExternalInput").ap()
        wf_d = nc.dram_tensor("wfT", [CR, 9 * C], f32, kind="ExternalInput").ap()
    fc_d = nc.dram_tensor("fcT", [128, 2 * C], f32, kind="ExternalInput").ap()
    b1_d = nc.dram_tensor("b1", [CR, 1], f32, kind="ExternalInput").ap()
    b2_d = nc.dram_tensor("b2", [CR, 1], f32, kind="ExternalInput").ap()
    bns_d = nc.dram_tensor("bns", [128, 2], f32, kind="ExternalInput").ap()
    bnh_d = nc.dram_tensor("bnh", [128, 2], f32, kind="ExternalInput").ap()
    fcb_d = nc.dram_tensor("fcb", [128, 2], f32, kind="ExternalInput").ap()
    out_d = nc.dram_tensor("out", [BL, C, HW], f32, kind="ExternalOutput").ap()

    Ident = mybir.ActivationFunctionType.Identity
    Relu = mybir.ActivationFunctionType.Relu
    Sigmoid = mybir.ActivationFunctionType.Sigmoid

    import contextlib

    with contextlib.ExitStack() as ctx:
        wpool = ctx.enter_context(tc.tile_pool(name="weights", bufs=1))
        xpool = ctx.enter_context(tc.tile_pool(name="x", bufs=2))
        padpool = ctx.enter_context(tc.tile_pool(name="pads", bufs=1))
        scrpool = ctx.enter_context(tc.tile_pool(name="scratch", bufs=2))
        smpool = ctx.enter_context(tc.tile_pool(name="sums", bufs=2))
        pp_small = ctx.enter_context(tc.tile_pool(name="ps", bufs=2, space="PSUM"))
        pp_big = ctx.enter_context(tc.tile_pool(name="pb", bufs=4, space="PSUM"))
        pp_fc = ctx.enter_context(tc.tile_pool(name="pfc", bufs=1, space="PSUM"))

        def load_weight(name, dram_ap, shape):
            t_f = wpool.tile(list(shape), f32, name=f"{name}_f32")
            nc.sync.dma_start(out=t_f[:], in_=dram_ap)
            if cdt == f32:
                return t_f
            t_c = wpool.tile(list(shape), cdt, name=f"{name}_c")
            nc.vector.tensor_copy(t_c[:], t_f[:])
            return t_c

        w1_sb = load_weight("w1", w1_d, (128, 2 * CR))
        if PACK:
            w2_sb = load_weight("w2", w2_d, (3 * CR, 3 * CR))
            wf_sb = load_weight("wf", wf_d, (3 * CR, 3 * C))
        else:
            w2_sb = load_weight("w2", w2_d, (CR, 9 * CR))
            wf_sb = load_weight("wf", wf_d, (CR, 9 * C))
        fc_sb = wpool.tile([128, 2 * C], f32)
        nc.sync.dma_start(out=fc_sb[:], in_=fc_d)
        b1_sb = wpool.tile([CR, 1], f32)
        nc.sync.dma_start(out=b1_sb[:], in_=b1_d)
        b2_sb = wpool.tile([CR, 1], f32)
        nc.sync.dma_start(out=b2_sb[:], in_=b2_d)
        bns_sb = wpool.tile([128, 2], f32)
        nc.sync.dma_start(out=bns_sb[:], in_=bns_d)
        bnh_sb = wpool.tile([128, 2], f32)
        nc.sync.dma_start(out=bnh_sb[:], in_=bnh_d)
        fcb_sb = wpool.tile([128, 2], f32)
        nc.sync.dma_start(out=fcb_sb[:], in_=fcb_d)

        # --- padded intermediates (borders stay zero across batches) ---
        # With PACK, partitions 0-31 hold the pad proper; partitions
        # 32-63 / 64-95 hold row-shifted replicas (pad[c, y+1], pad[c, y+2])
        # so 3 vertical taps contract in one K=96 matmul.
        pad_parts = 3 * CR if PACK else CR

        def zeroed_pad(name):
            t = padpool.tile([pad_parts, PADHW], cdt, name=name)
            if cdt == f32r:
                nc.vector.memset(t[:].bitcast(u32), 0)
            else:
                nc.vector.memset(t[:], 0.0)
            return t

        pad1 = zeroed_pad("pad1")
        pad2 = zeroed_pad("pad2")
        pad1_3d = pad1.rearrange("p (r c) -> p r c", r=H + 2)
        pad2_3d = pad2.rearrange("p (r c) -> p r c", r=H + 2)

        def replicate_pad(pad_flat):
            # replica dy: partitions 32dy..32dy+32, row t = pad row t+dy
            for dy in (1, 2):
                nrows = (H + 2) - dy
                nc.sync.dma_start(
                    out=pad_flat[32 * dy:32 * dy + CR, 0:nrows * PADW],
                    in_=pad_flat[0:CR, dy * PADW:(H + 2) * PADW],
                )

        for b in range(BL):
            # --- load x (both channel chunks) ---
            x_sb = []
            for cc in range(2):
                xt = xpool.tile([128, HW], f32, tag=f"x{cc}", name=f"x{cc}_{b}")
                nc.sync.dma_start(out=xt[:], in_=x_d[b, cc * 128:(cc + 1) * 128, :])
                x_sb.append(xt)

            if cdt == f32:
                xr = x_sb
            else:
                xr = []
                for cc in range(2):
                    xb = xpool.tile([128, HW], cdt, tag=f"xc{cc}", name=f"xc{cc}_{b}")
                    nc.vector.tensor_copy(xb[:], x_sb[cc][:])
                    xr.append(xb)

            # --- conv1: 1x1, 256 -> 32, write into pad1 interior ---
            for r in range(NCH):
                p1 = pp_small.tile([CR, 512], f32, tag="psmall", name=f"p1_{b}_{r}")
                for cc in range(2):
                    nc.tensor.matmul(
                        p1[:],
                        lhsT=w1_sb[:, cc * CR:(cc + 1) * CR],
                        rhs=xr[cc][:, r * 512:(r + 1) * 512],
                        start=(cc == 0),
                        stop=(cc == 1),
                    )
                dest = pad1_3d[0:CR, r * RPC + 1:r * RPC + 9, 1:W + 1]
                nc.scalar.activation(
                    dest, p1.rearrange("p (a b) -> p a b", a=RPC),
                    Ident, bias=b1_sb[:], scale=1.0,
                )
            if PACK:
                replicate_pad(pad1)

            # --- conv2: 3x3, 32 -> 32, write into pad2 interior ---
            if PACK:
                c2_taps = [(0, dx) for dx in range(3)]  # dy folded into K
            else:
                c2_taps = TAPS
            nt2 = len(c2_taps)
            for r in range(NCH):
                p2 = pp_small.tile([CR, 512], f32, tag="psmall", name=f"p2_{b}_{r}")
                for t, (dy, dx) in enumerate(c2_taps):
                    nc.tensor.matmul(
                        p2[:],
                        lhsT=w2_sb[:, t * CR:(t + 1) * CR],
                        rhs=pad1_3d[:, r * RPC + dy:r * RPC + dy + 8, dx:dx + W],
                        start=(t == 0),
                        stop=(t == nt2 - 1),
                    )
                dest = pad2_3d[0:CR, r * RPC + 1:r * RPC + 9, 1:W + 1]
                nc.scalar.activation(
                    dest, p2.rearrange("p (a b) -> p a b", a=RPC),
                    Ident, bias=b2_sb[:], scale=1.0,
                )
            if PACK:
                replicate_pad(pad2)

            # --- fused conv3+cbr: 3x3, 32 -> 256, BN+ReLU, row-sum accumulate ---
            epre = []
            for mc in range(2):
                sums = smpool.tile([128, NCH], f32, tag=f"sums{mc}",
                                   name=f"sums{mc}_{b}")
                for r in range(NCH):
                    pf = pp_big.tile([128, 512], f32, tag="pf", name=f"pf{mc}_{b}_{r}")
                    for t, (dy, dx) in enumerate(c2_taps):
                        nc.tensor.matmul(
                            pf[:],
                            lhsT=wf_sb[:, t * C + mc * 128:t * C + mc * 128 + 128],
                            rhs=pad2_3d[:, r * RPC + dy:r * RPC + dy + 8, dx:dx + W],
                            start=(t == 0),
                            stop=(t == nt2 - 1),
                        )
                    scr = scrpool.tile([128, 512], f32, tag="scr",
                                       name=f"scr{mc}_{b}_{r}")
                    nc.scalar.activation(
                        scr[:], pf[:], Relu,
                        bias=bnh_sb[:, mc:mc + 1], scale=bns_sb[:, mc:mc + 1],
                        accum_out=sums[:, r:r + 1],
                    )
                ep = smpool.tile([128, 1], f32, tag=f"epre{mc}", name=f"ep{mc}_{b}")
                nc.vector.tensor_reduce(
                    ep[:], sums[:], axis=mybir.AxisListType.X, op=mybir.AluOpType.add)
                epre.append(ep)

            # --- fc + sigmoid + gate ---
            for mc in range(2):
                pe = pp_fc.tile([128, 1], f32, tag="pe", name=f"pe{mc}_{b}")
                for k in range(2):
                    nc.tensor.matmul(
                        pe[:],
                        lhsT=fc_sb[:, k * C + mc * 128:k * C + mc * 128 + 128],
                        rhs=epre[k][:],
                        start=(k == 0),
                        stop=(k == 1),
                    )
                g = smpool.tile([128, 1], f32, tag=f"g{mc}", name=f"g{mc}_{b}")
                nc.scalar.activation(g[:], pe[:], Sigmoid,
                                     bias=fcb_sb[:, mc:mc + 1], scale=1.0)
                nc.vector.tensor_scalar_add(g[:], g[:], 1.0)
                nc.vector.tensor_scalar_mul(x_sb[mc][:], x_sb[mc][:], g[:])
                nc.sync.dma_start(out=out_d[b, mc * 128:(mc + 1) * 128, :],
                                  in_=x_sb[mc][:])


def _build_nc(mode):
    if mode in _NC_CACHE:
        return _NC_CACHE[mode]
    nc = bacc.Bacc("TRN2", target_bir_lowering=False, debug=False,
                   num_devices=N_CORES)
    with tile.TileContext(nc) as tc:
        _emit_kernel(tc, mode)
    nc.compile()
    _NC_CACHE[mode] = nc
    return nc


def _to_np(a, dtype=np.float32):
    return np.asarray(a, dtype=dtype)


def _prep_weights(inputs, mode):
    """Host-side weight folding. All arrays f32; on-chip casts as needed."""
    conv1_w = _to_np(inputs["conv1_w"], np.float64)  # [CR, C, 1, 1]
    conv1_b = _to_np(inputs["conv1_b"], np.float64)
    conv2_w = _to_np(inputs["conv2_w"], np.float64)  # [CR, CR, 3, 3]
    conv2_b = _to_np(inputs["conv2_b"], np.float64)
    conv3_w = _to_np(inputs["conv3_w"], np.float64)  # [C, CR, 1, 1]
    cbr_w = _to_np(inputs["cbr_w"], np.float64)      # [C, C, 3, 3]
    cbr_b = _to_np(inputs["cbr_b"], np.float64)
    bn_gamma = _to_np(inputs["bn_gamma"], np.float64)
    bn_beta = _to_np(inputs["bn_beta"], np.float64)
    bn_mean = _to_np(inputs["bn_mean"], np.float64)
    bn_var = _to_np(inputs["bn_var"], np.float64)
    fc_w = _to_np(inputs["fc_w"], np.float64)        # [C, C]
    fc_b = _to_np(inputs["fc_b"], np.float64)

    # conv1 lhsT: [K=256 (2x128 chunks), M=32] -> [128, 2*32]
    w1 = conv1_w.reshape(CR, C).T  # [C, CR]
    w1T = np.ascontiguousarray(
        w1.reshape(2, 128, CR).transpose(1, 0, 2).reshape(128, 2 * CR)
    ).astype(np.float32)

    c3 = conv3_w.reshape(C, CR)  # [m, i]
    if PACK:
        # K=96 rows are (dy, i); one column group per dx tap
        w2T = np.ascontiguousarray(
            conv2_w.transpose(2, 1, 3, 0).reshape(3 * CR, 3 * CR)
        ).astype(np.float32)  # [dy*32+i, dx*32+o]
        w_eff = np.einsum("omyx,mi->yixo", cbr_w, c3)  # [dy, i, dx, o]
        wfT = np.ascontiguousarray(
            w_eff.reshape(3 * CR, 3 * C)).astype(np.float32)
    else:
        # conv2 lhsT per tap: [i, (dy,dx), o] -> [32, 9*32]
        w2T = np.ascontiguousarray(
            conv2_w.transpose(1, 2, 3, 0).reshape(CR, 9 * CR)).astype(np.float32)
        w_eff = np.einsum("omyx,mi->iyxo", cbr_w, c3)  # [i, 3, 3, o]
        wfT = np.ascontiguousarray(w_eff.reshape(CR, 9 * C)).astype(np.float32)

    # BN fold: relu(psum * s + h)
    inv = 1.0 / np.sqrt(bn_var + BN_EPS)
    scale = bn_gamma * inv
    shift = (cbr_b - bn_mean) * scale + bn_beta
    bns = np.ascontiguousarray(scale.reshape(2, 128).T).astype(np.float32)
    bnh = np.ascontiguousarray(shift.reshape(2, 128).T).astype(np.float32)

    # fc lhsT with 1/K mean folded in: [K=256 (2x128), M=256] -> [128, 2*256]
    fcT = np.ascontiguousarray(
        (fc_w.T / K).reshape(2, 128, C).transpose(1, 0, 2).reshape(128, 2 * C)
    ).astype(np.float32)
    fcb = np.ascontiguousarray(fc_b.reshape(2, 128).T).astype(np.float32)

    return {
        "w1T": w1T,
        "w2T": w2T,
        "wfT": wfT,
        "fcT": fcT,
        "b1": conv1_b.reshape(CR, 1).astype(np.float32),
        "b2": conv2_b.reshape(CR, 1).astype(np.float32),
        "bns": bns,
        "bnh": bnh,
        "fcb": fcb,
    }


def _numpy_fallback(inputs):
    """Exact-path fallback (used only when conv3_b != 0, which the reference
    inputs never hit)."""
    x = _to_np(inputs["x"])  # [B, C, H, W]

    def conv1x1(xx, w, bias):
        co, ci = w.shape[0], w.shape[1]
        return (np.einsum("oi,bihw->bohw", w.reshape(co, ci), xx)
                + bias[None, :, None, None]).astype(np.float32)

    def conv3x3(xx, w, bias):
        bb, ci, hh, ww = xx.shape
        co = w.shape[0]
        xp = np.zeros((bb, ci, hh + 2, ww + 2), np.float32)
        xp[:, :, 1:-1, 1:-1] = xx
        y = np.zeros((bb, co, hh, ww), np.float32)
        for dy in range(3):
            for dx in range(3):
                y += np.einsum("oi,bihw->bohw", w[:, :, dy, dx],
                               xp[:, :, dy:dy + hh, dx:dx + ww])
        return y + bias[None, :, None, None]

    xe = conv1x1(x, _to_np(inputs["conv1_w"]), _to_np(inputs["conv1_b"]))
    xe = conv3x3(xe, _to_np(inputs["conv2_w"]), _to_np(inputs["conv2_b"]))
    xe = conv1x1(xe, _to_np(inputs["conv3_w"]), _to_np(inputs["conv3_b"]))
    xc = conv3x3(xe, _to_np(inputs["cbr_w"]), _to_np(inputs["cbr_b"]))
    inv = 1.0 / np.sqrt(_to_np(inputs["bn_var"]) + BN_EPS)
    xc = (xc - _to_np(inputs["bn_mean"])[None, :, None, None]) \
        * (_to_np(inputs["bn_gamma"]) * inv)[None, :, None, None] \
        + _to_np(inputs["bn_beta"])[None, :, None, None]
    xc = np.maximum(xc, 0.0)
    e_pre = xc.reshape(B, C, HW).sum(axis=2) / K  # softmax-sums-to-1 identity
    z = e_pre @ _to_np(inputs["fc_w"]).T + _to_np(inputs["fc_b"])
    e = 1.0 / (1.0 + np.exp(-z))
    return (x + x * e[:, :, None, None]).astype(np.float32)


def kernel(**inputs):
    if np.any(np.asarray(inputs["conv3_b"], np.float32)):
        return _numpy_fallback(inputs)

    mode = MODE
    x = _to_np(inputs["x"]).reshape(B, C, HW)
    weights = _prep_weights(inputs, mode)

    nc = _build_nc(mode)
    in_maps = []
    for c in range(N_CORES):
        m = dict(weights)
        m["x"] = np.ascontiguousarray(x[c * BL:(c + 1) * BL])
        in_maps.append(m)

    res = run_bass_kernel_spmd(nc, in_maps, list(range(N_CORES)))
    out = np.concatenate([res.results[c]["out"] for c in range(N_CORES)], axis=0)
    return out.reshape(B, C, H, W).astype(np.float32)


# revision 12
# speedup vs baseline: 1.9239x; 1.0421x over previous
"""Trainium2 Bass kernel for nn_Decoder_AdptiveVisualCenter_codebook.

Reference computation (B=16, C=256, H=W=64, CR=32, K=16):
    xe  = conv1x1(x, 256->32) ; conv3x3(32->32, pad 1) ; conv1x1(32->256)
    xc  = conv3x3(xe, 256->256, pad 1) ; BN(inference) ; ReLU
    xf  = xc as [b, n=4096, c]
    softmax-weighted codebook aggregation e_k = einsum(weights, xf) ; e_k.mean(1)
    e   = sigmoid(fc(e_k.mean(1)))
    out = x + x * e[:, :, None, None]

Two mathematical simplifications used here:
  1. softmax weights sum to 1 over K, so
         e_k.mean(axis=1)[b, c] = (1/K) * sum_n xf[b, n, c]
     -- the codebook / distances / softmax cancel out of the final output
     exactly (up to f32 rounding).
  2. conv3 (1x1) composes with the 3x3 cbr conv into a single 3x3 conv with
     Cin=32: W_eff[o,i,t] = sum_m cbr_w[o,m,t] * conv3_w[m,i].  This is exact
     when conv3_b == 0 (true for the reference inputs); a numpy fallback
     handles the general case.

Sharding: data-parallel over batch, 2 batch elements per core x 8 cores.
Weights are replicated (tiny).

Conv matmuls run in float32r (PE full rate, ~12-bit mantissa; measured
~17x more accurate than bf16).  f32r tiles must be produced by compute
ops (ACT/DVE round on write); raw DMA bits into an f32r matmul are
rejected by the BIR verifier and fault the hardware.
"""

import os
import sys

import numpy as np

for _p in ("/opt/trn_rl_repo",):
    if _p not in sys.path:
        sys.path.insert(0, _p)

from concourse import bacc, mybir, tile
import concourse.bass as bass
from concourse.bass_utils import run_bass_kernel_spmd

N_CORES = 8
B, C, H, W = 16, 256, 64, 64
HW = H * W
CR = 32
K = 16
BN_EPS = 1e-5
BL = B // N_CORES  # batches per core
PADW = W + 2  # 66
PADHW = PADW * (H + 2)  # 4356
NCH = 8  # n-chunks of 512 spatial positions (8 image rows each)
RPC = H // NCH  # rows per chunk = 8

# conv matmul dtype mode: 'f32r' (full-rate, ~12-bit mantissa),
# 'bf16' (full-rate, 8-bit), 'f32' (4x slower, exact)
MODE = os.environ.get("KERNEL_MODE", "f32r")
# pack 3 vertical conv taps into K=96 via row-shifted SBUF replicas
# (this box throttles the PE to 1.2 GHz under sustained matmul load, so
# wall time is proportional to total matmul columns -- fewer, K-fatter
# matmuls win)
PACK = os.environ.get("KERNEL_PACK", "1") == "1"

f32 = mybir.dt.float32
f32r = mybir.dt.float32r
bf16 = mybir.dt.bfloat16
u32 = mybir.dt.uint32

TAPS = [(dy, dx) for dy in range(3) for dx in range(3)]

_NC_CACHE = {}


def _emit_kernel(tc, mode):
    nc = tc.nc
    cdt = {"f32r": f32r, "bf16": bf16, "f32": f32}[mode]
    x_d = nc.dram_tensor("x", [BL, C, HW], f32, kind="ExternalInput").ap()
    w1_d = nc.dram_tensor("w1T", [128, 2 * CR], f32, kind="ExternalInput").ap()
    if PACK:
        w2_d = nc.dram_tensor("w2T", [3 * CR, 3 * CR], f32, kind="ExternalInput").ap()
        wf_d = nc.dram_tensor("wfT", [3 * CR, 3 * C], f32, kind="ExternalInput").ap()
    else:
        w2_d = nc.dram_tensor("w2T", [CR, 9 * CR], f32, kind="ExternalInput").ap()
        wf_d = nc.dram_tensor("wfT", [CR, 9 * C], f32, kind="ExternalInput").ap()
    fc_d = nc.dram_tensor("fcT", [128, 2 * C], f32, kind="ExternalInput").ap()
    b1_d = nc.dram_tensor("b1", [CR, 1], f32, kind="ExternalInput").ap()
    b2_d = nc.dram_tensor("b2", [CR, 1], f32, kind="ExternalInput").ap()
    bns_d = nc.dram_tensor("bns", [128, 2], f32, kind="ExternalInput").ap()
    bnh_d = nc.dram_tensor("bnh", [128, 2], f32, kind="ExternalInput").ap()
    fcb_d = nc.dram_tensor("fcb", [128, 2], f32, kind="ExternalInput").ap()
    out_d = nc.dram_tensor("out", [BL, C, HW], f32, kind="ExternalOutput").ap()

    Ident = mybir.ActivationFunctionType.Identity
    Relu = mybir.ActivationFunctionType.Relu
    Sigmoid = mybir.ActivationFunctionType.Sigmoid

    import contextlib

    with contextlib.ExitStack() as ctx:
        wpool = ctx.enter_context(tc.tile_pool(name="weights", bufs=1))
        xpool = ctx.enter_context(tc.tile_pool(name="x", bufs=2))
        padpool = ctx.enter_context(tc.tile_pool(name="pads", bufs=1))
        scrpool = ctx.enter_context(tc.tile_pool(name="scratch", bufs=2))
        smpool = ctx.enter_context(tc.tile_pool(name="sums", bufs=2))
        pp_small = ctx.enter_context(tc.tile_pool(name="ps", bufs=2, space="PSUM"))
        pp_big = ctx.enter_context(tc.tile_pool(name="pb", bufs=4, space="PSUM"))
        pp_fc = ctx.enter_context(tc.tile_pool(name="pfc", bufs=1, space="PSUM"))

        def load_weight(name, dram_ap, shape):
            t_f = wpool.tile(list(shape), f32, name=f"{name}_f32")
            nc.sync.dma_start(out=t_f[:], in_=dram_ap)
            if cdt == f32:
                return t_f
            t_c = wpool.tile(list(shape), cdt, name=f"{name}_c")
            nc.vector.tensor_copy(t_c[:], t_f[:])
            return t_c

        w1_sb = load_weight("w1", w1_d, (128, 2 * CR))
        if PACK:
            w2_sb = load_weight("w2", w2_d, (3 * CR, 3 * CR))
            wf_sb = load_weight("wf", wf_d, (3 * CR, 3 * C))
        else:
            w2_sb = load_weight("w2", w2_d, (CR, 9 * CR))
            wf_sb = load_weight("wf", wf_d, (CR, 9 * C))
        fc_sb = wpool.tile([128, 2 * C], f32)
        nc.sync.dma_start(out=fc_sb[:], in_=fc_d)
        b1_sb = wpool.tile([CR, 1], f32)
        nc.sync.dma_start(out=b1_sb[:], in_=b1_d)
        b2_sb = wpool.tile([CR, 1], f32)
        nc.sync.dma_start(out=b2_sb[:], in_=b2_d)
        bns_sb = wpool.tile([128, 2], f32)
        nc.sync.dma_start(out=bns_sb[:], in_=bns_d)
        bnh_sb = wpool.tile([128, 2], f32)
        nc.sync.dma_start(out=bnh_sb[:], in_=bnh_d)
        fcb_sb = wpool.tile([128, 2], f32)
        nc.sync.dma_start(out=fcb_sb[:], in_=fcb_d)

        # --- padded intermediates ---
        # With PACK, partitions 0-31 hold the pad proper; partitions
        # 32-63 / 64-95 hold row-shifted replicas (pad[c, y+1], pad[c, y+2])
        # so 3 vertical taps contract in one K=96 matmul.  Pads are
        # double-buffered (bufs=2) so the next batch's conv1 can overlap
        # this batch's later phases; only the borders need zeroing.
        pad_parts = 3 * CR if PACK else CR

        def zeroed_pad(tag, bno):
            t = padpool.tile([pad_parts, PADHW], cdt, tag=tag,
                             name=f"{tag}_{bno}")
            tz = t.bitcast(u32) if cdt == f32r else t
            z = 0 if cdt == f32r else 0.0
            t3z = tz.rearrange("p (r c) -> p r c", r=H + 2)
            nc.vector.memset(tz[0:CR, 0:PADW], z)                      # top row
            nc.vector.memset(tz[0:CR, (H + 1) * PADW:PADHW], z)       # bottom row
            nc.vector.memset(t3z[0:CR, 1:H + 1, 0:1], z)              # left col
            nc.vector.memset(t3z[0:CR, 1:H + 1, W + 1:W + 2], z)      # right col
            return t

        def replicate_pad(pad_flat):
            # replica dy: partitions 32dy..32dy+32, row t = pad row t+dy
            for dy in (1, 2):
                nrows = (H + 2) - dy
                nc.sync.dma_start(
                    out=pad_flat[32 * dy:32 * dy + CR, 0:nrows * PADW],
                    in_=pad_flat[0:CR, dy * PADW:(H + 2) * PADW],
                )

        # --- load + cast x for all batches up front (bufs=2 tags rotate);
        # split into halves so the first conv1 matmul starts early ---
        x_sb_all, xr_all = [], []
        for b in range(BL):
            x_sb, xr = [], []
            for cc in range(2):
                xt = xpool.tile([128, HW], f32, tag=f"x{cc}", name=f"x{cc}_{b}")
                for h in range(2):
                    nc.sync.dma_start(
                        out=xt[:, h * 2048:(h + 1) * 2048],
                        in_=x_d[b, cc * 128:(cc + 1) * 128, h * 2048:(h + 1) * 2048])
                x_sb.append(xt)
                if cdt == f32:
                    xr.append(xt)
                else:
                    xb = xpool.tile([128, HW], cdt, tag=f"xc{cc}", name=f"xc{cc}_{b}")
                    for h in range(2):
                        nc.vector.tensor_copy(xb[:, h * 2048:(h + 1) * 2048],
                                              xt[:, h * 2048:(h + 1) * 2048])
                    xr.append(xb)
            x_sb_all.append(x_sb)
            xr_all.append(xr)

        for b in range(BL):
            x_sb, xr = x_sb_all[b], xr_all[b]
            pad1 = zeroed_pad("pad1", b)
            pad2 = zeroed_pad("pad2", b)
            pad1_3d = pad1.rearrange("p (r c) -> p r c", r=H + 2)
            pad2_3d = pad2.rearrange("p (r c) -> p r c", r=H + 2)

            # --- conv1: 1x1, 256 -> 32, write into pad1 interior ---
            for r in range(NCH):
                p1 = pp_small.tile([CR, 512], f32, tag="psmall", name=f"p1_{b}_{r}")
                for cc in range(2):
                    nc.tensor.matmul(
                        p1[:],
                        lhsT=w1_sb[:, cc * CR:(cc + 1) * CR],
                        rhs=xr[cc][:, r * 512:(r + 1) * 512],
                        start=(cc == 0),
                        stop=(cc == 1),
                    )
                dest = pad1_3d[0:CR, r * RPC + 1:r * RPC + 9, 1:W + 1]
                nc.scalar.activation(
                    dest, p1.rearrange("p (a b) -> p a b", a=RPC),
                    Ident, bias=b1_sb[:], scale=1.0,
                )
            if PACK:
                replicate_pad(pad1)

            # --- conv2: 3x3, 32 -> 32, write into pad2 interior ---
            if PACK:
                c2_taps = [(0, dx) for dx in range(3)]  # dy folded into K
            else:
                c2_taps = TAPS
            nt2 = len(c2_taps)
            for r in range(NCH):
                p2 = pp_small.tile([CR, 512], f32, tag="psmall", name=f"p2_{b}_{r}")
                for t, (dy, dx) in enumerate(c2_taps):
                    nc.tensor.matmul(
                        p2[:],
                        lhsT=w2_sb[:, t * CR:(t + 1) * CR],
                        rhs=pad1_3d[:, r * RPC + dy:r * RPC + dy + 8, dx:dx + W],
                        start=(t == 0),
                        stop=(t == nt2 - 1),
                    )
                dest = pad2_3d[0:CR, r * RPC + 1:r * RPC + 9, 1:W + 1]
                nc.scalar.activation(
                    dest, p2.rearrange("p (a b) -> p a b", a=RPC),
                    Ident, bias=b2_sb[:], scale=1.0,
                )
            if PACK:
                replicate_pad(pad2)

            # --- fused conv3+cbr: 3x3, 32 -> 256, BN+ReLU, row-sum accumulate ---
            epre = []
            for mc in range(2):
                sums = smpool.tile([128, NCH], f32, tag=f"sums{mc}",
                                   name=f"sums{mc}_{b}")
                for r in range(NCH):
                    pf = pp_big.tile([128, 512], f32, tag="pf", name=f"pf{mc}_{b}_{r}")
                    for t, (dy, dx) in enumerate(c2_taps):
                        nc.tensor.matmul(
                            pf[:],
                            lhsT=wf_sb[:, t * C + mc * 128:t * C + mc * 128 + 128],
                            rhs=pad2_3d[:, r * RPC + dy:r * RPC + dy + 8, dx:dx + W],
                            start=(t == 0),
                            stop=(t == nt2 - 1),
                        )
                    scr = scrpool.tile([128, 512], f32, tag="scr",
                                       name=f"scr{mc}_{b}_{r}")
                    nc.scalar.activation(
                        scr[:], pf[:], Relu,
                        bias=bnh_sb[:, mc:mc + 1], scale=bns_sb[:, mc:mc + 1],
                        accum_out=sums[:, r:r + 1],
                    )
                ep = smpool.tile([128, 1], f32, tag=f"epre{mc}", name=f"ep{mc}_{b}")
                nc.vector.tensor_reduce(
                    ep[:], sums[:], axis=mybir.AxisListType.X, op=mybir.AluOpType.add)
                epre.append(ep)

            # --- fc + sigmoid + gate ---
            for mc in range(2):
                pe = pp_fc.tile([128, 1], f32, tag="pe", name=f"pe{mc}_{b}")
                for k in range(2):
                    nc.tensor.matmul(
                        pe[:],
                        lhsT=fc_sb[:, k * C + mc * 128:k * C + mc * 128 + 128],
                        rhs=epre[k][:],
                        start=(k == 0),
                        stop=(k == 1),
                    )
                g = smpool.tile([128, 1], f32, tag=f"g{mc}", name=f"g{mc}_{b}")
                nc.scalar.activation(g[:], pe[:], Sigmoid,
                                     bias=fcb_sb[:, mc:mc + 1], scale=1.0)
                nc.vector.tensor_scalar_add(g[:], g[:], 1.0)
                for h in range(2):
                    sl = slice(h * 2048, (h + 1) * 2048)
                    nc.vector.tensor_scalar_mul(x_sb[mc][:, sl], x_sb[mc][:, sl],
                                                g[:])
                    nc.sync.dma_start(
                        out=out_d[b, mc * 128:(mc + 1) * 128, sl],
                        in_=x_sb[mc][:, sl])


def _build_nc(mode):
    if mode in _NC_CACHE:
        return _NC_CACHE[mode]
    nc = bacc.Bacc("TRN2", target_bir_lowering=False, debug=False,
                   num_devices=N_CORES)
    with tile.TileContext(nc) as tc:
        _emit_kernel(tc, mode)
    nc.compile()
    _NC_CACHE[mode] = nc
    return nc


def _to_np(a, dtype=np.float32):
    return np.asarray(a, dtype=dtype)


def _prep_weights(inputs, mode):
    """Host-side weight folding. All arrays f32; on-chip casts as needed."""
    conv1_w = _to_np(inputs["conv1_w"], np.float64)  # [CR, C, 1, 1]
    conv1_b = _to_np(inputs["conv1_b"], np.float64)
    conv2_w = _to_np(inputs["conv2_w"], np.float64)  # [CR, CR, 3, 3]
    conv2_b = _to_np(inputs["conv2_b"], np.float64)
    conv3_w = _to_np(inputs["conv3_w"], np.float64)  # [C, CR, 1, 1]
    cbr_w = _to_np(inputs["cbr_w"], np.float64)      # [C, C, 3, 3]
    cbr_b = _to_np(inputs["cbr_b"], np.float64)
    bn_gamma = _to_np(inputs["bn_gamma"], np.float64)
    bn_beta = _to_np(inputs["bn_beta"], np.float64)
    bn_mean = _to_np(inputs["bn_mean"], np.float64)
    bn_var = _to_np(inputs["bn_var"], np.float64)
    fc_w = _to_np(inputs["fc_w"], np.float64)        # [C, C]
    fc_b = _to_np(inputs["fc_b"], np.float64)

    # conv1 lhsT: [K=256 (2x128 chunks), M=32] -> [128, 2*32]
    w1 = conv1_w.reshape(CR, C).T  # [C, CR]
    w1T = np.ascontiguousarray(
        w1.reshape(2, 128, CR).transpose(1, 0, 2).reshape(128, 2 * CR)
    ).astype(np.float32)

    c3 = conv3_w.reshape(C, CR)  # [m, i]
    if PACK:
        # K=96 rows are (dy, i); one column group per dx tap
        w2T = np.ascontiguousarray(
            conv2_w.transpose(2, 1, 3, 0).reshape(3 * CR, 3 * CR)
        ).astype(np.float32)  # [dy*32+i, dx*32+o]
        w_eff = np.einsum("omyx,mi->yixo", cbr_w, c3)  # [dy, i, dx, o]
        wfT = np.ascontiguousarray(
            w_eff.reshape(3 * CR, 3 * C)).astype(np.float32)
    else:
        # conv2 lhsT per tap: [i, (dy,dx), o] -> [32, 9*32]
        w2T = np.ascontiguousarray(
            conv2_w.transpose(1, 2, 3, 0).reshape(CR, 9 * CR)).astype(np.float32)
        w_eff = np.einsum("omyx,mi->iyxo", cbr_w, c3)  # [i, 3, 3, o]
        wfT = np.ascontiguousarray(w_eff.reshape(CR, 9 * C)).astype(np.float32)

    # BN fold: relu(psum * s + h)
    inv = 1.0 / np.sqrt(bn_var + BN_EPS)
    scale = bn_gamma * inv
    shift = (cbr_b - bn_mean) * scale + bn_beta
    bns = np.ascontiguousarray(scale.reshape(2, 128).T).astype(np.float32)
    bnh = np.ascontiguousarray(shift.reshape(2, 128).T).astype(np.float32)

    # fc lhsT with 1/K mean folded in: [K=256 (2x128), M=256] -> [128, 2*256]
    fcT = np.ascontiguousarray(
        (fc_w.T / K).reshape(2, 128, C).transpose(1, 0, 2).reshape(128, 2 * C)
    ).astype(np.float32)
    fcb = np.ascontiguousarray(fc_b.reshape(2, 128).T).astype(np.float32)

    return {
        "w1T": w1T,
        "w2T": w2T,
        "wfT": wfT,
        "fcT": fcT,
        "b1": conv1_b.reshape(CR, 1).astype(np.float32),
        "b2": conv2_b.reshape(CR, 1).astype(np.float32),
        "bns": bns,
        "bnh": bnh,
        "fcb": fcb,
    }


def _numpy_fallback(inputs):
    """Exact-path fallback (used only when conv3_b != 0, which the reference
    inputs never hit)."""
    x = _to_np(inputs["x"])  # [B, C, H, W]

    def conv1x1(xx, w, bias):
        co, ci = w.shape[0], w.shape[1]
        return (np.einsum("oi,bihw->bohw", w.reshape(co, ci), xx)
                + bias[None, :, None, None]).astype(np.float32)

    def conv3x3(xx, w, bias):
        bb, ci, hh, ww = xx.shape
        co = w.shape[0]
        xp = np.zeros((bb, ci, hh + 2, ww + 2), np.float32)
        xp[:, :, 1:-1, 1:-1] = xx
        y = np.zeros((bb, co, hh, ww), np.float32)
        for dy in range(3):
            for dx in range(3):
                y += np.einsum("oi,bihw->bohw", w[:, :, dy, dx],
                               xp[:, :, dy:dy + hh, dx:dx + ww])
        return y + bias[None, :, None, None]

    xe = conv1x1(x, _to_np(inputs["conv1_w"]), _to_np(inputs["conv1_b"]))
    xe = conv3x3(xe, _to_np(inputs["conv2_w"]), _to_np(inputs["conv2_b"]))
    xe = conv1x1(xe, _to_np(inputs["conv3_w"]), _to_np(inputs["conv3_b"]))
    xc = conv3x3(xe, _to_np(inputs["cbr_w"]), _to_np(inputs["cbr_b"]))
    inv = 1.0 / np.sqrt(_to_np(inputs["bn_var"]) + BN_EPS)
    xc = (xc - _to_np(inputs["bn_mean"])[None, :, None, None]) \
        * (_to_np(inputs["bn_gamma"]) * inv)[None, :, None, None] \
        + _to_np(inputs["bn_beta"])[None, :, None, None]
    xc = np.maximum(xc, 0.0)
    e_pre = xc.reshape(B, C, HW).sum(axis=2) / K  # softmax-sums-to-1 identity
    z = e_pre @ _to_np(inputs["fc_w"]).T + _to_np(inputs["fc_b"])
    e = 1.0 / (1.0 + np.exp(-z))
    return (x + x * e[:, :, None, None]).astype(np.float32)


def kernel(**inputs):
    if np.any(np.asarray(inputs["conv3_b"], np.float32)):
        return _numpy_fallback(inputs)

    mode = MODE
    x = _to_np(inputs["x"]).reshape(B, C, HW)
    weights = _prep_weights(inputs, mode)

    nc = _build_nc(mode)
    in_maps = []
    for c in range(N_CORES):
        m = dict(weights)
        m["x"] = np.ascontiguousarray(x[c * BL:(c + 1) * BL])
        in_maps.append(m)

    res = run_bass_kernel_spmd(nc, in_maps, list(range(N_CORES)))
    out = np.concatenate([res.results[c]["out"] for c in range(N_CORES)], axis=0)
    return out.reshape(B, C, H, W).astype(np.float32)
